# revision 26
# baseline (speedup 1.0000x reference)
"""LongcatFlashTopkRouter on 8 Trainium2 NeuronCores.

Math (per token t):
    logits = h_t @ W.T                      # [768]
    s      = softmax(logits)
    c      = s + bias                       # bias-corrected selection scores
    idx    = top12(c)                       # descending, ties -> lower index
    w      = 2.5 * s[idx] / sum(s[idx])

Device-side reformulation (per token, no softmax materialization needed):
    e   = exp(logits)           (no max-subtraction: |logits| < ~9 is safe in fp32)
    se  = sum(e)
    z   = e + se * bias         # z has the SAME ordering as c = e/se + bias
    top-16 of z -> (z16, idx16) via DVE max/max_index/match_replace
Host epilogue (cheap, vectorized numpy):
    e16 = z16 - se * bias[idx16]
    w   = 2.5 * e16[:, :12] / sum(e16[:, :12])   # the 1/se factor cancels

Sharding: tokens (batch*seq = 32768) split evenly across 8 cores (4096 each);
W and bias replicated. Hidden states are pre-transposed on the host into
[tile, k-partition, k-chunk, token] layout so each 128-token tile's 16
contraction chunks are contiguous SBUF-ready [128, 2048] blocks.

Matmul runs in float32r (full-rate fp32 PE mode).
"""

import numpy as np

import concourse.bass as bass
import concourse.mybir as mybir
from concourse import bacc
from concourse.tile import TileContext
from concourse.bass_utils import run_bass_kernel_spmd

N_CORES = 8
B, S, H, E = 4, 8192, 2048, 768
TOK = B * S // N_CORES      # 4096 tokens per core
TT = 32                     # token tiles of 128 per core
KC = H // 128               # 16 contraction chunks
TOPK = 12
TOP16 = 16
SCALE = 2.5

F32 = mybir.dt.float32
F32R = mybir.dt.float32r
U32 = mybir.dt.uint32
EXP = mybir.ActivationFunctionType.Exp
MULT = mybir.AluOpType.mult
ADD = mybir.AluOpType.add


WT_SPLIT = 4  # wt loaded in 4 chunk-DMAs so tile-0 matmuls start early


def build_nc(mm_dtype=F32R):
    nc = bacc.Bacc()
    ht = nc.dram_tensor("ht", [TT, 128, KC, 128], mm_dtype, kind="ExternalInput")
    wt = nc.dram_tensor("wt", [128, KC, E], mm_dtype, kind="ExternalInput")
    biasb = nc.dram_tensor("biasb", [128, E], F32, kind="ExternalInput")
    # packed per-tile output: [z16 f32 | idx16 u32-bits | sumexp f32]
    o_pack = nc.dram_tensor("o_pack", [TT, 128, 33], F32, kind="ExternalOutput")

    with TileContext(nc) as tc:
        with (
            tc.tile_pool(name="const", bufs=1) as cpool,
            tc.tile_pool(name="hin", bufs=4) as hpool,
            tc.tile_pool(name="mid", bufs=2) as mpool,
            tc.tile_pool(name="small", bufs=3) as spool,
            tc.tile_pool(name="ps", bufs=3, space="PSUM") as ppool,
        ):
            # Resident router weight [k_in_chunk, chunk*expert] and bias rows.
            # wt is pipelined per k-chunk, alternating between the two HWDGE
            # queues (sync/scalar), so tile-0 matmuls start after ~1 chunk
            # instead of after the full 6 MB load.
            h0 = hpool.tile([128, KC * 128], mm_dtype, tag="h")
            nc.sync.dma_start(out=h0, in_=ht[0])
            # one tile per k-chunk so a matmul only waits on its own chunk's DMA
            wt_sb = []
            for c in range(KC):
                wtc = cpool.tile([128, E], mm_dtype, tag=f"wt{c}")
                eng = nc.sync if c % 2 == 0 else nc.scalar
                eng.dma_start(out=wtc, in_=wt[:, c])
                wt_sb.append(wtc)
            bias_sb = cpool.tile([128, E], F32)
            nc.gpsimd.dma_start(out=bias_sb, in_=biasb[:])

            for t in range(TT):
                if t == 0:
                    h_sb = h0
                else:
                    h_sb = hpool.tile([128, KC * 128], mm_dtype, tag="h")
                    nc.sync.dma_start(out=h_sb, in_=ht[t])

                # logits[j_token, e] accumulated over 16 k-chunks; two PSUM
                # bank regions (512 + 256 fp32).
                ps = ppool.tile([128, E], F32, tag="ps")
                for c in range(KC):
                    lhsT = h_sb[:, c * 128:(c + 1) * 128]
                    nc.tensor.matmul(
                        ps[:, 0:512], lhsT, wt_sb[c][:, 0:512],
                        start=(c == 0), stop=(c == KC - 1),
                    )
                    nc.tensor.matmul(
                        ps[:, 512:E], lhsT, wt_sb[c][:, 512:E],
                        start=(c == 0), stop=(c == KC - 1),
                    )

                # packed result tile: z16 | idx16 | se
                comb = spool.tile([128, 33], F32, tag="comb")
                se = comb[:, 32:33]

                # e = exp(logits), se = rowsum(e)  (ScalarE, single pass)
                ez = mpool.tile([128, E], F32, tag="ez")
                nc.scalar.activation(out=ez, in_=ps, func=EXP, accum_out=se)

                # z = bias * se + e — scaled-bias on ScalarE (AP scale),
                # add on the otherwise-idle GpSimd; keeps DVE free for top-k.
                br = mpool.tile([128, E], F32, tag="br")
                nc.scalar.activation(
                    out=br, in_=bias_sb, func=mybir.ActivationFunctionType.Copy,
                    scale=se,
                )
                z = mpool.tile([128, E], F32, tag="z")
                nc.gpsimd.tensor_add(z, ez, br)

                # top-16 (values + indices), descending
                i16 = comb[:, 16:32].bitcast(U32)
                z2 = mpool.tile([128, E], F32, tag="z2")
                nc.vector.max(comb[:, 0:8], z)
                nc.vector.max_index(i16[:, 0:8], comb[:, 0:8], z)
                nc.vector.match_replace(z2, comb[:, 0:8], z, imm_value=-1.0)
                nc.vector.max(comb[:, 8:16], z2)
                nc.vector.max_index(i16[:, 8:16], comb[:, 8:16], z2)

                nc.scalar.dma_start(out=o_pack[t], in_=comb)
    nc.finalize()
    return nc


def _prep_inputs(h, W_, b):
    # [k_in_chunk(p), chunk(c), expert(e)]: wtprep[p, c, e] = W[e, c*128 + p]
    wtprep = np.ascontiguousarray(W_.T.reshape(KC, 128, E).transpose(1, 0, 2))
    biasb = np.ascontiguousarray(np.broadcast_to(b, (128, E)))
    in_maps = []
    for core in range(N_CORES):
        hc = h[core * TOK:(core + 1) * TOK]
        # [tile, token_in_tile(j), chunk(c), k_in_chunk(p)] -> [tile, p, c, j]
        h4 = hc.reshape(TT, 128, KC, 128)
        htp = np.ascontiguousarray(h4.transpose(0, 3, 2, 1))
        in_maps.append({"ht": htp, "wt": wtprep, "biasb": biasb})
    return in_maps


RISK_TAU = 1e-3  # relative z-gap below which f32r noise could flip ordering


def _epilogue(results, b, h_flat, W):
    idx_list, w_list, risk_list = [], [], []
    for r in results:
        pack = np.ascontiguousarray(r["o_pack"].reshape(-1, 33))
        z16 = pack[:, 0:16]
        idx16 = pack[:, 16:32].view(np.uint32)
        se = pack[:, 32:33]
        e16 = (z16 - se * b[idx16]).astype(np.float32)
        e12 = e16[:, :TOPK]
        denom = e12.sum(axis=-1, keepdims=True, dtype=np.float32) + np.float32(1e-20) * se
        w_list.append((np.float32(SCALE) * e12 / denom).astype(np.float32))
        idx_list.append(idx16[:, :TOPK].astype(np.int32))
        # flag tokens whose adjacent top-16 gaps are inside the f32r noise band
        gaps = (z16[:, :-1] - z16[:, 1:]) / np.abs(z16[:, :1])
        risk_list.append(gaps.min(axis=-1) < RISK_TAU)
    topk_idx = np.concatenate(idx_list, axis=0)
    topk_w = np.concatenate(w_list, axis=0)

    # fp32-exact host recompute for at-risk tokens (mimics the reference op
    # sequence exactly in float32)
    risk = np.concatenate(risk_list, axis=0)
    ridx = np.nonzero(risk)[0]
    if ridx.size:
        lg = h_flat[ridx] @ W.T.astype(np.float32)
        mx = lg.max(axis=-1, keepdims=True)
        ex = np.exp(lg - mx)
        s = ex / ex.sum(axis=-1, keepdims=True, dtype=np.float32)
        c = s + b
        ii = np.argsort(-c, axis=-1, kind="stable")[:, :TOPK]
        ww = np.take_along_axis(s, ii, axis=-1)
        ww = ww / (ww.sum(axis=-1, keepdims=True, dtype=np.float32) + np.float32(1e-20))
        topk_idx[ridx] = ii.astype(np.int32)
        topk_w[ridx] = (np.float32(SCALE) * ww).astype(np.float32)

    topk_idx = topk_idx.reshape(B, S, TOPK)
    topk_w = topk_w.reshape(B, S, TOPK).astype(np.float32)
    return topk_idx, topk_w


_NC_CACHE = {}


def run(hidden_states, W, e_score_correction_bias, trace=False, mm_dtype=F32R):
    key = (str(mm_dtype),)
    if key not in _NC_CACHE:
        _NC_CACHE[key] = build_nc(mm_dtype)
    nc = _NC_CACHE[key]
    h = np.ascontiguousarray(np.asarray(hidden_states, dtype=np.float32)).reshape(-1, H)
    W_ = np.ascontiguousarray(np.asarray(W, dtype=np.float32))
    b = np.ascontiguousarray(np.asarray(e_score_correction_bias, dtype=np.float32))
    in_maps = _prep_inputs(h, W_, b)
    res = run_bass_kernel_spmd(nc, in_maps, core_ids=list(range(N_CORES)), trace=trace)
    out = _epilogue(res.results, b, h, W_)
    return out, res


def kernel(hidden_states, W, e_score_correction_bias):
    out, _ = run(hidden_states, W, e_score_correction_bias, trace=False)
    return out


# revision 28
# speedup vs baseline: 1.1332x; 1.1332x over previous
"""LongcatFlashTopkRouter on 8 Trainium2 NeuronCores.

Math (per token t):
    logits = h_t @ W.T                      # [768]
    s      = softmax(logits)
    c      = s + bias                       # bias-corrected selection scores
    idx    = top12(c)                       # descending, ties -> lower index
    w      = 2.5 * s[idx] / sum(s[idx])

Device-side reformulation (per token, no softmax materialization needed):
    e   = exp(logits)           (no max-subtraction: |logits| < ~9 is safe in fp32)
    se  = sum(e)
    z   = e + se * bias         # z has the SAME ordering as c = e/se + bias
    top-16 of z -> (z16, idx16) via DVE max/max_index/match_replace
Host epilogue (cheap, vectorized numpy):
    e16 = z16 - se * bias[idx16]
    w   = 2.5 * e16[:, :12] / sum(e16[:, :12])   # the 1/se factor cancels

Sharding: tokens (batch*seq = 32768) split evenly across 8 cores (4096 each);
W and bias replicated. Hidden states are pre-transposed on the host into
[tile, k-partition, k-chunk, token] layout so each 128-token tile's 16
contraction chunks are contiguous SBUF-ready [128, 2048] blocks.

Matmul runs in float32r (full-rate fp32 PE mode).
"""

import numpy as np

import concourse.bass as bass
import concourse.mybir as mybir
from concourse import bacc
from concourse.tile import TileContext
from concourse.bass_utils import run_bass_kernel_spmd

N_CORES = 8
B, S, H, E = 4, 8192, 2048, 768
TOK = B * S // N_CORES      # 4096 tokens per core
TT = 32                     # token tiles of 128 per core
KC = H // 128               # 16 contraction chunks
TOPK = 12
TOP16 = 16
SCALE = 2.5

F32 = mybir.dt.float32
F32R = mybir.dt.float32r
U32 = mybir.dt.uint32
EXP = mybir.ActivationFunctionType.Exp
MULT = mybir.AluOpType.mult
ADD = mybir.AluOpType.add


WT_SPLIT = 4  # wt loaded in 4 chunk-DMAs so tile-0 matmuls start early


def build_nc(mm_dtype=F32R):
    nc = bacc.Bacc()
    ht = nc.dram_tensor("ht", [TT, 128, KC, 128], mm_dtype, kind="ExternalInput")
    wt = nc.dram_tensor("wt", [128, KC, E], mm_dtype, kind="ExternalInput")
    biasb = nc.dram_tensor("biasb", [128, E], F32, kind="ExternalInput")
    # packed per-tile output: [z16 f32 | idx16 u32-bits | sumexp f32]
    o_pack = nc.dram_tensor("o_pack", [TT, 128, 33], F32, kind="ExternalOutput")

    with TileContext(nc) as tc:
        with (
            tc.tile_pool(name="const", bufs=1) as cpool,
            tc.tile_pool(name="hin", bufs=4) as hpool,
            tc.tile_pool(name="mid", bufs=2) as mpool,
            tc.tile_pool(name="small", bufs=3) as spool,
            tc.tile_pool(name="ps", bufs=3, space="PSUM") as ppool,
        ):
            # Resident router weight [k_in_chunk, chunk*expert] and bias rows.
            # wt is pipelined per k-chunk, alternating between the two HWDGE
            # queues (sync/scalar), so tile-0 matmuls start after ~1 chunk
            # instead of after the full 6 MB load.
            # First PRO_T h-tiles load up front; then the 16 wt chunks stream
            # in (alternating HWDGE queues). The prologue runs those tiles
            # CHUNK-MAJOR so the PE does useful work on each wt chunk as it
            # arrives instead of idling for the whole 6 MB wt load.
            PRO_T = 3
            h_tiles = {}
            for t in range(PRO_T):
                h_t = hpool.tile([128, KC * 128], mm_dtype, tag="h")
                eng = nc.sync if t % 2 == 0 else nc.scalar
                eng.dma_start(out=h_t, in_=ht[t])
                h_tiles[t] = h_t
            # one tile per k-chunk so a matmul only waits on its own chunk
            wt_sb = []
            for c in range(KC):
                wtc = cpool.tile([128, E], mm_dtype, tag=f"wt{c}")
                eng = nc.sync if c % 2 == 0 else nc.scalar
                eng.dma_start(out=wtc, in_=wt[:, c])
                wt_sb.append(wtc)
            bias_sb = cpool.tile([128, E], F32)
            nc.gpsimd.dma_start(out=bias_sb, in_=biasb[:])

            def mm_tile(h_sb, ps, c):
                lhsT = h_sb[:, c * 128:(c + 1) * 128]
                nc.tensor.matmul(
                    ps[:, 0:512], lhsT, wt_sb[c][:, 0:512],
                    start=(c == 0), stop=(c == KC - 1),
                )
                nc.tensor.matmul(
                    ps[:, 512:E], lhsT, wt_sb[c][:, 512:E],
                    start=(c == 0), stop=(c == KC - 1),
                )

            def post_tile(t, ps):
                # packed result tile: z16 | idx16 | se
                comb = spool.tile([128, 33], F32, tag="comb")
                se = comb[:, 32:33]

                # e = exp(logits), se = rowsum(e)  (ScalarE, single pass)
                ez = mpool.tile([128, E], F32, tag="ez")
                nc.scalar.activation(out=ez, in_=ps, func=EXP, accum_out=se)

                # z = bias * se + e — scaled-bias on ScalarE (AP scale),
                # add on the otherwise-idle GpSimd; keeps DVE free for top-k.
                br = mpool.tile([128, E], F32, tag="br")
                nc.scalar.activation(
                    out=br, in_=bias_sb, func=mybir.ActivationFunctionType.Copy,
                    scale=se,
                )
                z = mpool.tile([128, E], F32, tag="z")
                nc.gpsimd.tensor_add(z, ez, br)

                # top-16 (values + indices), descending
                i16 = comb[:, 16:32].bitcast(U32)
                z2 = mpool.tile([128, E], F32, tag="z2")
                nc.vector.max(comb[:, 0:8], z)
                nc.vector.max_index(i16[:, 0:8], comb[:, 0:8], z)
                nc.vector.match_replace(z2, comb[:, 0:8], z, imm_value=-1.0)
                nc.vector.max(comb[:, 8:16], z2)
                nc.vector.max_index(i16[:, 8:16], comb[:, 8:16], z2)

                nc.scalar.dma_start(out=o_pack[t], in_=comb)

            # chunk-major prologue over the first PRO_T tiles
            ps_pro = [
                ppool.tile([128, E], F32, tag="ps", name=f"ps_pro{i}")
                for i in range(PRO_T)
            ]
            for c in range(KC):
                for t in range(PRO_T):
                    mm_tile(h_tiles[t], ps_pro[t], c)
            for t in range(PRO_T):
                post_tile(t, ps_pro[t])

            # steady state: tile-major
            for t in range(PRO_T, TT):
                h_sb = hpool.tile([128, KC * 128], mm_dtype, tag="h")
                nc.sync.dma_start(out=h_sb, in_=ht[t])
                ps = ppool.tile([128, E], F32, tag="ps")
                for c in range(KC):
                    mm_tile(h_sb, ps, c)
                post_tile(t, ps)
    nc.finalize()
    return nc


def _prep_inputs(h, W_, b):
    # [k_in_chunk(p), chunk(c), expert(e)]: wtprep[p, c, e] = W[e, c*128 + p]
    wtprep = np.ascontiguousarray(W_.T.reshape(KC, 128, E).transpose(1, 0, 2))
    biasb = np.ascontiguousarray(np.broadcast_to(b, (128, E)))
    in_maps = []
    for core in range(N_CORES):
        hc = h[core * TOK:(core + 1) * TOK]
        # [tile, token_in_tile(j), chunk(c), k_in_chunk(p)] -> [tile, p, c, j]
        h4 = hc.reshape(TT, 128, KC, 128)
        htp = np.ascontiguousarray(h4.transpose(0, 3, 2, 1))
        in_maps.append({"ht": htp, "wt": wtprep, "biasb": biasb})
    return in_maps


RISK_TAU = 1e-3  # relative z-gap below which f32r noise could flip ordering


def _epilogue(results, b, h_flat, W):
    idx_list, w_list, risk_list = [], [], []
    for r in results:
        pack = np.ascontiguousarray(r["o_pack"].reshape(-1, 33))
        z16 = pack[:, 0:16]
        idx16 = pack[:, 16:32].view(np.uint32)
        se = pack[:, 32:33]
        e16 = (z16 - se * b[idx16]).astype(np.float32)
        e12 = e16[:, :TOPK]
        denom = e12.sum(axis=-1, keepdims=True, dtype=np.float32) + np.float32(1e-20) * se
        w_list.append((np.float32(SCALE) * e12 / denom).astype(np.float32))
        idx_list.append(idx16[:, :TOPK].astype(np.int32))
        # flag tokens whose adjacent top-16 gaps are inside the f32r noise band
        gaps = (z16[:, :-1] - z16[:, 1:]) / np.abs(z16[:, :1])
        risk_list.append(gaps.min(axis=-1) < RISK_TAU)
    topk_idx = np.concatenate(idx_list, axis=0)
    topk_w = np.concatenate(w_list, axis=0)

    # fp32-exact host recompute for at-risk tokens (mimics the reference op
    # sequence exactly in float32)
    risk = np.concatenate(risk_list, axis=0)
    ridx = np.nonzero(risk)[0]
    if ridx.size:
        lg = h_flat[ridx] @ W.T.astype(np.float32)
        mx = lg.max(axis=-1, keepdims=True)
        ex = np.exp(lg - mx)
        s = ex / ex.sum(axis=-1, keepdims=True, dtype=np.float32)
        c = s + b
        ii = np.argsort(-c, axis=-1, kind="stable")[:, :TOPK]
        ww = np.take_along_axis(s, ii, axis=-1)
        ww = ww / (ww.sum(axis=-1, keepdims=True, dtype=np.float32) + np.float32(1e-20))
        topk_idx[ridx] = ii.astype(np.int32)
        topk_w[ridx] = (np.float32(SCALE) * ww).astype(np.float32)

    topk_idx = topk_idx.reshape(B, S, TOPK)
    topk_w = topk_w.reshape(B, S, TOPK).astype(np.float32)
    return topk_idx, topk_w


_NC_CACHE = {}


def run(hidden_states, W, e_score_correction_bias, trace=False, mm_dtype=F32R):
    key = (str(mm_dtype),)
    if key not in _NC_CACHE:
        _NC_CACHE[key] = build_nc(mm_dtype)
    nc = _NC_CACHE[key]
    h = np.ascontiguousarray(np.asarray(hidden_states, dtype=np.float32)).reshape(-1, H)
    W_ = np.ascontiguousarray(np.asarray(W, dtype=np.float32))
    b = np.ascontiguousarray(np.asarray(e_score_correction_bias, dtype=np.float32))
    in_maps = _prep_inputs(h, W_, b)
    res = run_bass_kernel_spmd(nc, in_maps, core_ids=list(range(N_CORES)), trace=trace)
    out = _epilogue(res.results, b, h, W_)
    return out, res


def kernel(hidden_states, W, e_score_correction_bias):
    out, _ = run(hidden_states, W, e_score_correction_bias, trace=False)
    return out
